# revision 1
# baseline (speedup 1.0000x reference)
"""CRF loss (forward-algorithm partition function minus gold score, batch mean)
on 8 Trainium2 NeuronCores.

Strategy: pure data parallel over batch (512 -> 64 per core), plus a 16-way
SEQUENCE split per core to break the serial recurrence.

Per-core math (exp-space reformulation of the log-space recurrence):
    e_{s+1} = exp(feat_s) * (M @ e_s),   M[n,p] = exp(trans[n,p] - c)
Products of positive matrices contract to rank-1 exponentially fast, so the
1024-step chain is split into 16 segments of 64 steps. Each segment's chain
starts DELTA=2 steps early from a uniform vector ("warmup"): after 2 steps
the state direction matches the true forward message far below the fp8 input
noise floor (validated in numpy); only the scale is off. Scales are stitched
with column-sum ratios:
    log z = log(stop . u_15) + sum_k [log t_{k-1} - log h_k]
where t_k = colsum of chain k's final state and h_k = colsum of chain k's
state at warmup end. Chain 0 needs no warmup (it is reset to the exact
e_start at round DELTA).

The 16 chains run as 2 GANGS of 8 (states bf16, 64 tags on partitions x
512 (chain, batch) columns, ping-ponging partition halves by round parity).
Per gang per round: one 512-wide bf16 matmul (PE, via tile_position
quadrants) and one 512-wide DVE tensor_tensor multiply draining PSUM (DVE is
the only engine that can both read PSUM and multiply two tensors; GPSIMD
cannot touch PSUM, and quadrant-alternating PSUM accumulation groups hang
the PE). The two gangs interleave on DVE to hide each other's ~1.1us serial
round latency. exp(feat) runs on the Activation engine over 2-round blocks issued 4+
blocks ahead, reading the host-pre-transposed fp8 feats buffer (tag-major,
fp8e4m3 -- validated: the loss error from fp8 feats is ~3.5e-5 relative),
so there are no device-side DMA transposes and the DMA head is ~12us.

Gold score (device-side, off the critical path):
  - emit: sum feats[b,s,tag[b,s]] via ONE-HOT DIAG MATMULS on the otherwise
    idle PE: the host ships an fp8 one-hot(tags) tensor in the same
    tag-major layout; per chunk, 64 accumulating fp8 matmuls compute
    out[b,b'] += sum_n feat[n,b]*onehot[n,b'] (two fixed-quadrant PSUM
    accumulation groups, one per step parity) and a DVE scalar_tensor_tensor
    against an identity mask extracts and accumulates the diagonal.
  - transitions: sum trans[cur,prev] = <histogram(cur*64+prev), trans>; the
    host counts index pairs (pure index prep), the device does one
    scalar_tensor_tensor dot product against the lane-replicated table.
"""

import numpy as np
import ml_dtypes
from contextlib import ExitStack

import concourse.bass as bass
import concourse.tile as tile
from concourse import bacc, mybir
from concourse.bass_utils import run_bass_kernel_spmd

F32 = mybir.dt.float32
F8 = mybir.dt.float8e4
BF16 = mybir.dt.bfloat16
U16 = mybir.dt.uint16

B, S, T = 512, 1024, 64
NCORES = 8
BS = B // NCORES          # 64 batches per core
START_TAG, STOP_TAG = 62, 63
CSHIFT = 5.1              # per-step constant log shift folded into M

K = 16                    # sequence segments (= 64-step chunks)
L = S // K                # 64 steps per segment
DELTA = 2                 # warmup steps per chain (even)
R = L + DELTA             # 72 rounds
G = 8                     # chains per gang
NG = K // G               # 2 gangs
GW = G * BS               # 512 cols per gang
CCH = (L // 2) * BS       # 2048 cols per chunk in featT
NCHUNK = K
CH_DVE = 8                # chains per gang multiplied on DVE (all of them)
EMW = 16 * BS             # 1024: emit gather stream width per chunk


def crf_kernel(ctx: ExitStack, tc: tile.TileContext, outs, ins):
    nc = tc.nc
    (lnt_o, redh_o, esum_o, tsum_o) = outs
    (featT_i, transT_i, stopcol_i, initcol_i, table_i, counts_i,
     oh8_i, eye_i) = ins

    const = ctx.enter_context(tc.tile_pool(name="const", bufs=1))
    efp = ctx.enter_context(tc.tile_pool(name="ef", bufs=3))
    qp = [ctx.enter_context(tc.tile_pool(name=f"q{g}", bufs=2, space="PSUM"))
          for g in range(NG)]
    zp = ctx.enter_context(tc.tile_pool(name="z", bufs=1, space="PSUM"))
    dqp = ctx.enter_context(tc.tile_pool(name="dq", bufs=1, space="PSUM"))
    smp = ctx.enter_context(tc.tile_pool(name="sm", bufs=2))

    # ---- constants ----
    mtraw = const.tile([128, T], F32)
    nc.sync.dma_start(mtraw[0:64, :], transT_i[:, :])
    nc.sync.dma_start(mtraw[64:128, :], transT_i[:, :])
    negc = const.tile([128, 1], F32)
    nc.vector.memset(negc[:, :], -CSHIFT)
    mt = const.tile([128, T], BF16)   # exp(trans.T - c), both halves, bf16
    nc.scalar.activation(mt[:, :], mtraw[:, :],
                         mybir.ActivationFunctionType.Exp, bias=negc[:, :])

    stopraw = const.tile([128, 1], F32)
    nc.sync.dma_start(stopraw[64:128, :], stopcol_i[:, :])
    stopt = const.tile([128, 1], BF16)
    nc.scalar.activation(stopt[64:128, :], stopraw[64:128, :],
                         mybir.ActivationFunctionType.Exp)

    ones_col = const.tile([128, 1], BF16)
    nc.vector.memset(ones_col[:, :], 1.0)
    initcol = const.tile([128, BS], BF16)  # e_start pattern in rows 64:128
    nc.sync.dma_start(initcol[64:128, :], initcol_i[:, :])


    # ---- feats (tag-major bf16, host-transposed), streamed per chunk ----
    featT = const.tile([128, (NCHUNK + 1) * CCH], F8)
    nc.vector.memset(featT[:, 0:CCH], 0.0)           # warmup chunk: feat = 0
    for c in range(1, NCHUNK + 1, 4):
        nc.sync.dma_start(featT[:, c * CCH:(c + 4) * CCH],
                          featT_i[:, (c - 1) * CCH:(c + 3) * CCH])
    featT3 = featT[:, :].rearrange("p (c x) -> p c x", x=CCH)

    table = const.tile([128, 256], F32)
    nc.sync.dma_start(table[:, :], table_i[:, :])
    counts = const.tile([128, 256], BF16)
    nc.sync.dma_start(counts[:, :], counts_i[:, :])
    eye = const.tile([T, T], BF16)
    nc.sync.dma_start(eye[:, :], eye_i[:, :])

    oh8 = const.tile([128, NCHUNK * CCH], F8)
    for c in range(NCHUNK):
        nc.sync.dma_start(oh8[:, c * CCH:(c + 1) * CCH],
                          oh8_i[:, c * CCH:(c + 1) * CCH])

    # ---- per-gang state [128, 512]; one DVE multiply per gang per round
    #      (DVE is the only engine that can both read PSUM and do tensor x
    #      tensor; the two gangs interleave to hide each other's latency).
    sts = []
    for g in range(NG):
        st = const.tile([128, GW], BF16, tag=f"st{g}")
        nc.vector.memset(st[0:64, :], 1.0 / T)
        nc.vector.memset(st[64:128, :], 1.0 / T)
        sts.append(st)

    # log-scale stash: [h(16 chains) | t(16 chains)] x 64 batches
    zbuf = const.tile([1, 2 * K * BS], F32)

    esums = const.tile([128, NCHUNK], F32)
    nc.vector.memset(esums[:, :], 0.0)
    red_h = const.tile([1, BS], F32)
    lnt = const.tile([1, K * BS], F32)

    NBLK = R // 2
    ef_blks = [[None] * NBLK for _ in range(NG)]

    def issue_exp(g, m):
        # ef block (2 rounds): exp of the 8 chains' 1-column (64-elem) window
        eb = efp.tile([128, G * BS], BF16, tag=f"ef{g}_{m % 8}")
        r0 = 2 * m
        off = 0 if r0 < DELTA else 1
        j0 = (L - DELTA + r0) // 2 if r0 < DELTA else (r0 - DELTA) // 2
        i0 = G * g + off
        src3 = featT3[:, i0:i0 + G, j0 * BS:(j0 + 1) * BS]
        nc.scalar.activation(eb[:, :].rearrange("p (i x) -> p i x", x=BS),
                             src3, mybir.ActivationFunctionType.Exp)
        ef_blks[g][m] = eb

    dq_cur = [None, None]

    def issue_emit_part(c, part):
        # emit via one-hot diag matmuls: accumulating fp8 matmuls, then a
        # diag-extract. One accumulation group per step-parity: the PE
        # cannot alternate tile_position quadrants within a group.
        if c == 0 and part == 0:
            dq_e = dqp.tile([64, T], F32, tag="dqe")
            dq_o = dqp.tile([64, T], F32, tag="dqo")
            dq_cur[0], dq_cur[1] = dq_e, dq_o
        for k in range(16):
            s_in = part * 16 + k
            hs, j = s_in % 2, s_in // 2
            col0 = (c + 1) * CCH + j * BS
            ohcol0 = c * CCH + j * BS
            nc.tensor.matmul(dq_cur[hs][:, :],
                             featT[hs * 64:hs * 64 + 64, col0:col0 + BS],
                             oh8[hs * 64:hs * 64 + 64, ohcol0:ohcol0 + BS],
                             start=(c == 0 and s_in == hs),
                             stop=(c == NCHUNK - 1 and s_in >= 62),
                             tile_position=(hs * 64, 0))
        if c == NCHUNK - 1 and part == 3:
            for hs in (0, 1):
                dsc = smp.tile([64, T], BF16, tag=f"dsc{hs}")
                nc.vector.scalar_tensor_tensor(
                    dsc[:, :], dq_cur[hs][:, :], 1.0, eye[:, :],
                    op0=mybir.AluOpType.mult, op1=mybir.AluOpType.mult,
                    accum_out=esums[hs * 64:hs * 64 + 64, 0:1])

    def capture(g, dst_off):
        # column-sums of gang g's 8 chains' states into zbuf
        z = zp.tile([1, GW], F32, tag=f"z{g}")
        nc.tensor.matmul(z[:, :], ones_col[64:128, :],
                         sts[g][64:128, :], tile_position=(64, 0))
        nc.vector.tensor_copy(
            zbuf[0:1, dst_off + g * GW:dst_off + (g + 1) * GW], z[:, :])

    # ---- main loop ----
    # Gang 0 only needs featT chunks 1..8 (in HBM-arrival order) so it is
    # issued LEAD rounds ahead of gang 1 and starts while gang 1's chunks
    # are still streaming in.
    LEAD = 0

    def gang_round(g, r):
        hs = r % 2
        hr = 1 - hs
        m = r // 2
        la = 8 if g == 0 else 4
        if r % 2 == 0 and m + la < NBLK:
            issue_exp(g, m + la)
        if r == DELTA:
            # h-capture at position 64k (state after round DELTA-1, half 1)
            capture(g, 0)
            if g == 0:
                # chain-0 reset to the exact e_start
                nc.scalar.copy(sts[0][64:128, 0:BS], initcol[64:128, :])
        st = sts[g]
        ebv = ef_blks[g][m][hs * 64:hs * 64 + 64, :].rearrange(
            "p (i b) -> p i b", b=BS)
        q = qp[g].tile([128, GW], F32)
        nc.tensor.matmul(q[hs * 64:hs * 64 + 64, :],
                         mt[hr * 64:hr * 64 + 64, :],
                         st[hr * 64:hr * 64 + 64, :],
                         tile_position=(hr * 64, hs * 64))
        nc.vector.tensor_tensor(
            st[hs * 64:hs * 64 + 64, :].rearrange("p (i b) -> p i b", b=BS),
            q[hs * 64:hs * 64 + 64, :].rearrange("p (i b) -> p i b", b=BS),
            ebv[:, :, :],
            op=mybir.AluOpType.mult)

    for m in range(8):
        issue_exp(0, m)
    for m in range(4):
        issue_exp(1, m)
    for r0 in range(LEAD):
        gang_round(0, r0)
    for r in range(R + LEAD):
        if r < R:
            gang_round(1, r)
        if LEAD + r < R:
            gang_round(0, LEAD + r)
        if r == 20:
            # h-part of the stitching (captured at round DELTA) done early;
            # also warms the Ln activation table while Act has slack
            lnh = smp.tile([1, (K - 1) * BS], F32, tag="lnh")
            nc.scalar.activation(lnh[:, :], zbuf[0:1, BS:K * BS],
                                 mybir.ActivationFunctionType.Ln)
            nc.vector.tensor_reduce(
                red_h[:, :],
                lnh[0:1, :].rearrange("p (k b) -> p b k", b=BS),
                axis=mybir.AxisListType.X, op=mybir.AluOpType.add)
            nc.sync.dma_start(redh_o[:, :], red_h[:, :])
        if r >= 2 and (r - 2) // 4 < NCHUNK:
            issue_emit_part((r - 2) // 4, (r - 2) % 4)

    # gold transitions: one dot product against the replicated table
    tsc = const.tile([128, 256], F32)
    tsum = const.tile([128, 1], F32)
    nc.vector.scalar_tensor_tensor(tsc[:, :], table[:, :], 1.0, counts[:, :],
                                   op0=mybir.AluOpType.mult,
                                   op1=mybir.AluOpType.mult,
                                   accum_out=tsum[:, :])
    nc.sync.dma_start(tsum_o[:, :], tsum[:, :])

    # ---- t-capture: colsums stay in PSUM; Act takes Ln straight from there
    for g in range(NG):
        z = zp.tile([1, GW], F32, tag=f"z{g}")
        nc.tensor.matmul(z[:, :], ones_col[64:128, :],
                         sts[g][64:128, :], tile_position=(64, 0))
        w = GW if g == 0 else GW - BS      # chain 15's slot comes from stop
        nc.scalar.activation(lnt[0:1, g * GW:g * GW + w], z[0:1, 0:w],
                             mybir.ActivationFunctionType.Ln)
    # chain 15 terminal: stop . state (reuses z1's bank)
    tq = zp.tile([1, GW], F32, tag="z1")
    nc.tensor.matmul(tq[0:1, 0:BS], stopt[64:128, :],
                     sts[1][64:128, GW - BS:GW],
                     tile_position=(64, 0))
    nc.scalar.activation(lnt[0:1, K * BS - BS:K * BS], tq[0:1, 0:BS],
                         mybir.ActivationFunctionType.Ln)

    # fwd[b] = sum_k lnt[k,b] - red_h[b] is finished on the host
    nc.sync.dma_start(lnt_o[:, :], lnt[:, :])

    esum = const.tile([128, 1], F32)
    nc.vector.tensor_reduce(esum[:, :], esums[:, :],
                            axis=mybir.AxisListType.X, op=mybir.AluOpType.add)
    nc.sync.dma_start(esum_o[:, :], esum[:, :])


def build():
    nc = bacc.Bacc("TRN2", target_bir_lowering=False, debug=False)
    ins_spec = [
        ("featT", [128, NCHUNK * CCH], F8),
        ("transT", [T, T], F32),
        ("stopcol", [T, 1], F32),
        ("initcol", [T, BS], BF16),
        ("table", [128, 256], F32),
        ("counts", [128, 256], BF16),
        ("oh8", [128, NCHUNK * CCH], F8),
        ("eye", [T, T], BF16),
    ]
    outs_spec = [
        ("lnt", [1, K * BS], F32),
        ("redh", [1, BS], F32),
        ("esum", [128, 1], F32),
        ("tsum", [128, 1], F32),
    ]
    ins = [nc.declare_dram_parameter(n, s, d, isOutput=False).ap()
           for n, s, d in ins_spec]
    outs = [nc.declare_dram_parameter(n, s, d, isOutput=True).ap()
            for n, s, d in outs_spec]
    with tile.TileContext(nc) as tc:
        with ExitStack() as ctx:
            crf_kernel(ctx, tc, outs, ins)
    nc.compile()
    return nc


def host_prep(feats, transitions, tags, mask):
    """Build the 8 per-core input maps + host-side pad corrections."""
    assert feats.shape == (B, S, T) and transitions.shape == (T, T)
    mask_arr = np.asarray(mask)
    assert np.all(mask_arr == 1), "kernel assumes an all-ones mask"
    feats = np.asarray(feats, dtype=np.float32)
    transitions = np.asarray(transitions, dtype=np.float32)
    tags = np.asarray(tags).astype(np.int64)

    transT = np.ascontiguousarray(transitions.T)
    stopcol = np.ascontiguousarray(transitions[STOP_TAG, :].reshape(T, 1))
    initcol = np.zeros((T, BS), ml_dtypes.bfloat16)
    initcol[START_TAG, :] = 1.0
    tflat = transitions.reshape(4096)
    table = np.zeros((128, 256), np.float32)
    p_ = np.arange(128)
    table[:, :] = tflat[(p_[:, None] % 16) + 16 * np.arange(256)[None, :]]

    eye = np.eye(T, dtype=ml_dtypes.bfloat16)

    in_maps = []
    for core in range(NCORES):
        b0 = core * BS
        fb = feats[b0:b0 + BS].astype(ml_dtypes.float8_e4m3fn)  # (64,1024,64)
        tg = tags[b0:b0 + BS]

        # featT[(s%2)*64+n, c*2048 + ((s%64)//2)*64 + b] = fb[b, s, n]
        x = fb.reshape(BS, K, L // 2, 2, T)                 # (b, c, j, hs, n)
        featT = np.ascontiguousarray(
            x.transpose(3, 4, 1, 2, 0).reshape(128, NCHUNK * CCH))

        # one-hot tags in the same tag-major layout (for the emit diag-mms)
        ohsrc = np.zeros((BS, S, T), ml_dtypes.float8_e4m3fn)
        np.put_along_axis(ohsrc, tg[:, :, None], ml_dtypes.float8_e4m3fn(1.0),
                          axis=2)
        xo = ohsrc.reshape(BS, K, L // 2, 2, T)
        oh8 = np.ascontiguousarray(
            xo.transpose(3, 4, 1, 2, 0).reshape(128, NCHUNK * CCH))

        # transition-pair histogram (bf16-exact small counts)
        cur = np.concatenate([tg, np.full((BS, 1), STOP_TAG, np.int64)], 1)
        prev = np.concatenate([np.full((BS, 1), START_TAG, np.int64), tg], 1)
        lin = (cur * T + prev).reshape(-1)
        cnt = np.bincount(lin, minlength=4096)
        assert cnt.max() < 256
        counts = np.zeros((128, 256), ml_dtypes.bfloat16)
        counts[0:16, :] = cnt.reshape(256, 16).T

        in_maps.append({
            "featT": featT, "transT": transT, "stopcol": stopcol,
            "initcol": initcol, "table": table, "counts": counts,
            "oh8": oh8, "eye": eye,
        })
    return in_maps


def host_finish(results):
    fwd_total = 0.0
    gold_total = 0.0
    for r in results:
        fwd_total += (float(r["lnt"].astype(np.float64).sum())
                      - float(r["redh"].astype(np.float64).sum())
                      + BS * S * CSHIFT)
        gold_total += float(r["esum"].astype(np.float64).sum())
        gold_total += float(r["tsum"].astype(np.float64).sum())
    return np.asarray((fwd_total - gold_total) / B, dtype=np.float32)


_NC = None


def kernel(feats, transitions, tags, mask):
    global _NC
    if _NC is None:
        _NC = build()
    in_maps = host_prep(feats, transitions, tags, mask)
    res = run_bass_kernel_spmd(_NC, in_maps, list(range(NCORES)))
    return host_finish(res.results)


if __name__ == "__main__":
    import reference
    inp = reference.setup_inputs()
    out = kernel(**{k: np.asarray(v) for k, v in inp.items()})
    print("kernel loss:", out)

